# revision 13
# baseline (speedup 1.0000x reference)
"""RWKV-4 block (TimeMix WKV recurrence + ChannelMix) on 8 Trainium2 NeuronCores.

Sharding: 8 cores = 4 batch elements x 2 halves of T (1024 output rows each).
All compute is row-local except the WKV scan carry state, which is passed
between half-pairs with one tiny AllGather ([128, 2*DG] fp32 per core).

Device layout: channels-on-partitions [c, t].  The host pre-transposes x and
all weight matrices so every DMA is contiguous.  WKV runs as a hardware
tensor_tensor_scan (state = ew*state + x) per 128-channel group.  Large
intermediates (ek, ek*v, sigmoid(r), x2, sigmoid(r2)) are spilled to DRAM and
streamed back so SBUF tile-pool lifetimes nest (stack allocator).
"""

import os
import sys

import numpy as np

for _p in ("/opt/trn_rl_repo", "/root/.axon_site/_ro/trn_rl_repo"):
    if os.path.isdir(_p) and _p not in sys.path:
        sys.path.insert(0, _p)

import ml_dtypes  # noqa: E402

import concourse.bass as bass  # noqa: E402,F401
import concourse.mybir as mybir  # noqa: E402
import concourse.tile as tile  # noqa: E402
from concourse import bacc  # noqa: E402
from concourse.bass_utils import run_bass_kernel_spmd  # noqa: E402

F32 = mybir.dt.float32
F32R = mybir.dt.float32r
BF16 = mybir.dt.bfloat16
ALU = mybir.AluOpType
ACT = mybir.ActivationFunctionType

B, T, C, D_ATT, D_FFN = 4, 2048, 2048, 2048, 8192
EPS = 1e-5
N_CORES = 8
DEN_EPS = 1e-30  # keeps Den finite on the virtual row of first-half cores


def _splits(total, sz):
    return [(s, min(sz, total - s)) for s in range(0, total, sz)]


def _even_splits(total, mx):
    n = -(-total // mx)
    base, rem = divmod(total, n)
    out, s = [], 0
    for i in range(n):
        sz = base + (1 if i < rem else 0)
        out.append((s, sz))
        s += sz
    return out


def build_program(Cc=C, Dd=D_ATT, Ff=D_FFN, rows_out=T // 2, n_cores=N_CORES,
                  mm_dt=BF16, no_collective=False):
    """Build the (uniform SPMD) Bass program."""
    P = 128
    CG, DG, FG = Cc // P, Dd // P, Ff // P
    RO = rows_out              # output rows per core
    RS = RO + 1                # scan rows (one leading row)
    R = RS + 1                 # loaded x rows (two leading rows)
    NV = 11

    nc = bacc.Bacc("TRN2", target_bir_lowering=False, debug=False,
                   num_devices=n_cores)

    xT = nc.dram_tensor("xT", [Cc, R], F32, kind="ExternalInput").ap()
    wk = nc.dram_tensor("wk", [P, CG, Dd], mm_dt, kind="ExternalInput").ap()
    wv = nc.dram_tensor("wv", [P, CG, Dd], mm_dt, kind="ExternalInput").ap()
    wr = nc.dram_tensor("wr", [P, CG, Dd], mm_dt, kind="ExternalInput").ap()
    wo = nc.dram_tensor("wo", [P, DG, Cc], mm_dt, kind="ExternalInput").ap()
    wck = nc.dram_tensor("wck", [P, CG, Ff], mm_dt, kind="ExternalInput").ap()
    wcv = nc.dram_tensor("wcv", [P, FG, Cc], mm_dt, kind="ExternalInput").ap()
    wcr = nc.dram_tensor("wcr", [P, CG, Cc], mm_dt, kind="ExternalInput").ap()
    cvec = nc.dram_tensor("cvec", [P, CG, NV], F32, kind="ExternalInput").ap()
    m0d = nc.dram_tensor("m0", [P, 1], F32, kind="ExternalInput").ap()
    seld = nc.dram_tensor("sel", [P, n_cores], F32, kind="ExternalInput").ap()
    outT = nc.dram_tensor("outT", [Cc, RO], F32, kind="ExternalOutput").ap()

    xTv = xT.rearrange("(g p) r -> p g r", p=P)
    outTv = outT.rearrange("(g p) r -> p g r", p=P)

    I_LN1W, I_LN1B, I_TMK, I_TMV, I_TMR, I_EW, I_EU, I_LN2W, I_LN2B, \
        I_CMK, I_CMR = range(NV)

    TS = 512                 # matmul moving free-dim tile
    LTS = min(256, RS)       # layernorm streaming tile

    with tile.TileContext(nc) as tc:
        const = tc.alloc_tile_pool(name="const", bufs=1)
        con = const.tile([P, CG, NV], F32, tag="con")
        nc.sync.dma_start(out=con[:], in_=cvec)
        m0 = const.tile([P, 1], F32, tag="m0")
        nc.sync.dma_start(out=m0[:], in_=m0d)
        selt = const.tile([P, n_cores], F32, tag="sel")
        nc.sync.dma_start(out=selt[:], in_=seld)
        onesc = const.tile([P, 1], F32, tag="ones")
        nc.vector.memset(onesc[:], 1.0)
        onesb = const.tile([P, 1], BF16, tag="onesb")
        nc.vector.memset(onesb[:], 1.0)
        epsc = const.tile([1, 1], F32, tag="epsc")
        nc.vector.memset(epsc[:], EPS)
        onesP = const.tile([1, P], F32, tag="onesP")
        nc.vector.memset(onesP[:], 1.0)

        def ccol(g, i):
            return con[:, g, i:i + 1]

        dram = tc.alloc_tile_pool(name="dram", bufs=1, space="DRAM")
        ekdram = dram.tile([Dd, RS], BF16)
        ekdv = ekdram.rearrange("(g p) r -> p g r", p=P)
        xkvdram = dram.tile([Dd, RS], BF16)
        xkvdv = xkvdram.rearrange("(g p) r -> p g r", p=P)
        srdram = dram.tile([Dd, RS], BF16)
        srdv = srdram.rearrange("(g p) r -> p g r", p=P)
        x2dram = dram.tile([Cc, RS], F32)
        x2dv = x2dram.rearrange("(g p) r -> p g r", p=P)
        sgdram = dram.tile([Cc, RO], BF16)
        sgdv = sgdram.rearrange("(g p) r -> p g r", p=P)
        cc_in = dram.tile([P, 2 * DG], F32)
        cc_out = dram.tile([P * n_cores, 2 * DG], F32)

        # ---- LayerNorm over partition-dim channels, streaming from DRAM ----
        def ln_stream(src_v, nrows, iw, out_sb, name):
            """src_v: DRAM view [P, CG, nrows] fp32 -> out_sb [P,CG,nrows] bf16."""
            st = tc.alloc_tile_pool(name=f"{name}_st", bufs=1)
            sp = tc.alloc_tile_pool(name=f"{name}_sp", bufs=2)
            psum = tc.alloc_tile_pool(name=f"{name}_ps", bufs=2, space="PSUM")
            ssum = st.tile([1, nrows], F32, tag="sum", name="ssum")
            ssq = st.tile([1, nrows], F32, tag="sq", name="ssq")
            for t0, tsz in _splits(nrows, LTS):
                xls = sp.tile([P, CG, LTS], F32, tag="xls", name="xls")
                nc.sync.dma_start(out=xls[:, :, :tsz],
                                  in_=src_v[:, :, t0:t0 + tsz])
                xsq = sp.tile([P, CG, LTS], BF16, tag="lnsq", name="xsq")
                nc.scalar.activation(xsq[:, :, :tsz], xls[:, :, :tsz],
                                     ACT.Square)
                xbf = sp.tile([P, CG, LTS], BF16, tag="lnbf", name="xbf")
                nc.vector.tensor_copy(out=xbf[:, :, :tsz],
                                      in_=xls[:, :, :tsz])
                ps = psum.tile([1, LTS], F32, tag="ln_ps", name="ps")
                ps2 = psum.tile([1, LTS], F32, tag="ln_ps2", name="ps2")
                for g in range(CG):
                    nc.tensor.matmul(
                        ps[:, :tsz], onesb[:], xbf[:, g, :tsz],
                        start=(g == 0), stop=(g == CG - 1))
                    nc.tensor.matmul(
                        ps2[:, :tsz], onesb[:], xsq[:, g, :tsz],
                        start=(g == 0), stop=(g == CG - 1))
                nc.vector.tensor_copy(out=ssum[:, t0:t0 + tsz],
                                      in_=ps[:, :tsz])
                nc.vector.tensor_copy(out=ssq[:, t0:t0 + tsz],
                                      in_=ps2[:, :tsz])
            mu = st.tile([1, nrows], F32, tag="mu", name="mu")
            rstd = st.tile([1, nrows], F32, tag="rstd", name="rstd")
            var = st.tile([1, nrows], F32, tag="var", name="var")
            musq = st.tile([1, nrows], F32, tag="musq", name="musq")
            nc.vector.tensor_scalar_mul(mu[:], ssum[:], 1.0 / Cc)
            nc.vector.tensor_scalar_mul(var[:], ssq[:], 1.0 / Cc)
            nc.vector.tensor_tensor(musq[:], mu[:], mu[:], ALU.mult)
            nc.vector.tensor_tensor(var[:], var[:], musq[:], ALU.subtract)
            nc.scalar.activation(var[:], var[:], ACT.Ln, bias=epsc[:])
            nc.scalar.activation(rstd[:], var[:], ACT.Exp, scale=-0.5)
            for t0, tsz in _splits(nrows, LTS):
                xls = sp.tile([P, CG, LTS], F32, tag="xls", name="xls")
                nc.sync.dma_start(out=xls[:, :, :tsz],
                                  in_=src_v[:, :, t0:t0 + tsz])
                # broadcast per-row stats to all 128 partitions via K=1 matmul
                mups = psum.tile([P, LTS], F32, tag="mups", name="mups")
                nc.tensor.matmul(mups[:, :tsz], onesP[:],
                                 mu[:, t0:t0 + tsz],
                                 start=True, stop=True)
                rsps = psum.tile([P, LTS], F32, tag="rsps", name="rsps")
                nc.tensor.matmul(rsps[:, :tsz], onesP[:],
                                 rstd[:, t0:t0 + tsz],
                                 start=True, stop=True)
                for g in range(CG):
                    xm = sp.tile([P, LTS], F32, tag="ln_xm", name="xm")
                    nc.vector.tensor_tensor(xm[:, :tsz], xls[:, g, :tsz],
                                            mups[:, :tsz], ALU.subtract)
                    nc.vector.scalar_tensor_tensor(
                        out_sb[:, g, t0:t0 + tsz], xm[:, :tsz], ccol(g, iw),
                        rsps[:, :tsz], ALU.mult, ALU.mult)
            for p in (psum, sp, st):
                p.release()

        # ================= Phase A: LN1 =================
        pHs = tc.alloc_tile_pool(name="pHs", bufs=1)
        hs = pHs.tile([P, CG, R], BF16, tag="hs")
        ln_stream(xTv, R, I_LN1W, hs, "ln1")
        # zero the two lead rows on first-half cores (time_shift zero pad)
        nc.vector.tensor_scalar_mul(hs[:, :, 0:2], hs[:, :, 0:2], m0[:])

        # ============ Phase B: mixes + k/v/r matmuls ============
        pMix = tc.alloc_tile_pool(name="pMix", bufs=2)
        wpB = tc.alloc_tile_pool(name="wpB", bufs=2)
        stg = tc.alloc_tile_pool(name="stg", bufs=3)
        psB = tc.alloc_tile_pool(name="psB", bufs=3, space="PSUM")

        DBLK = min(512, Dd)

        def make_mix(icoef):
            mix = pMix.tile([P, CG, RS], BF16, tag="mix", name="mix")
            for g in range(CG):
                dmix = stg.tile([P, RS], BF16, tag="dmix", name="dmix")
                nc.vector.tensor_tensor(dmix[:], hs[:, g, 1:R],
                                        hs[:, g, 0:RS], ALU.subtract)
                nc.vector.scalar_tensor_tensor(
                    mix[:, g, :], dmix[:], ccol(g, icoef), hs[:, g, 0:RS],
                    ALU.mult, ALU.add)
            return mix

        def mm_phase(wdram, rhs, n_out, nrows, evict):
            for d0, dsz in _splits(n_out, DBLK):
                wbuf = wpB.tile([P, CG, DBLK], mm_dt, tag="w3", name="w3")
                nc.sync.dma_start(out=wbuf[:, :, :dsz],
                                  in_=wdram[:, :, d0:d0 + dsz])
                for gl in range(dsz // P):
                    g_out = (d0 + gl * P) // P
                    for t0, tsz in _even_splits(nrows, TS):
                        ps = psB.tile([P, TS], F32, tag="mm_ps", name="mm_ps")
                        for gi in range(CG):
                            nc.tensor.matmul(
                                ps[:, :tsz],
                                wbuf[:, gi, gl * P:(gl + 1) * P],
                                rhs[:, gi, t0:t0 + tsz],
                                start=(gi == 0), stop=(gi == CG - 1))
                        evict(g_out, t0, tsz, ps)

        def evict_k(g, t0, tsz, ps):
            est = stg.tile([P, TS], BF16, tag="est", name="est")
            nc.scalar.activation(est[:, :tsz], ps[:, :tsz], ACT.Exp)
            if t0 == 0:  # mask the virtual lead row on first-half cores
                nc.vector.tensor_scalar_mul(est[:, 0:1], est[:, 0:1], m0[:])
            nc.sync.dma_start(out=ekdv[:, g, t0:t0 + tsz], in_=est[:, :tsz])

        def evict_v(g, t0, tsz, ps):
            eld = stg.tile([P, TS], BF16, tag="eld", name="eld")
            nc.sync.dma_start(out=eld[:, :tsz], in_=ekdv[:, g, t0:t0 + tsz])
            xst = stg.tile([P, TS], BF16, tag="xst", name="xst")
            nc.vector.tensor_tensor(xst[:, :tsz], eld[:, :tsz], ps[:, :tsz],
                                    ALU.mult)
            nc.sync.dma_start(out=xkvdv[:, g, t0:t0 + tsz], in_=xst[:, :tsz])

        def evict_r(g, t0, tsz, ps):
            srt = stg.tile([P, TS], BF16, tag="srt", name="srt")
            nc.scalar.activation(srt[:, :tsz], ps[:, :tsz], ACT.Sigmoid)
            nc.sync.dma_start(out=srdv[:, g, t0:t0 + tsz], in_=srt[:, :tsz])

        mixk = make_mix(I_TMK)
        mm_phase(wk, mixk, Dd, RS, evict_k)
        mixv = make_mix(I_TMV)
        mm_phase(wv, mixv, Dd, RS, evict_v)
        mixr = make_mix(I_TMR)
        mm_phase(wr, mixr, Dd, RS, evict_r)

        psB.release()
        stg.release()
        wpB.release()
        pMix.release()
        pHs.release()

        # ============ Phase C: boundary states + AllGather ============
        # Right-side pool: C's DVE scans overlap phase B's matmuls without
        # waiting on B's pool-zone releases.
        pC = tc.alloc_tile_pool(name="pC", bufs=2, side="right")

        state = pC.tile([P, 2 * DG], F32, tag="state", name="state")
        for g in range(DG):
            ekg = pC.tile([P, RS], BF16, tag="ekg", name="ekg")
            nc.sync.dma_start(out=ekg[:], in_=ekdv[:, g, :])
            xkg = pC.tile([P, RS], BF16, tag="xkg", name="xkg")
            nc.sync.dma_start(out=xkg[:], in_=xkvdv[:, g, :])
            ewbc = ccol(g, I_EW).to_broadcast([P, RS - 1])
            apre = pC.tile([P, RS - 1], F32, tag="apre", name="apre")
            nc.vector.tensor_tensor_scan(
                apre[:], ewbc, xkg[:, :RS - 1], 0.0, ALU.mult, ALU.add)
            nc.gpsimd.tensor_copy(out=state[:, g:g + 1],
                                  in_=apre[:, RS - 2:RS - 1])
            bpre = pC.tile([P, RS - 1], F32, tag="bpre", name="bpre")
            nc.vector.tensor_tensor_scan(
                bpre[:], ewbc, ekg[:, :RS - 1], 0.0, ALU.mult, ALU.add)
            nc.gpsimd.tensor_copy(out=state[:, DG + g:DG + g + 1],
                                  in_=bpre[:, RS - 2:RS - 1])
        nc.sync.dma_start(out=cc_in[:], in_=state[:])
        if not no_collective:
            nc.gpsimd.collective_compute(
                "AllGather", ALU.bypass,
                replica_groups=[list(range(n_cores))],
                ins=[cc_in[:].opt()], outs=[cc_out[:].opt()])
        else:  # timing-equivalent stand-in for TimelineSim profiling
            for jj in range(n_cores):
                nc.sync.dma_start(out=cc_out[jj * P:(jj + 1) * P, :],
                                  in_=cc_in[:])
        gsb = pC.tile([P, n_cores, 2 * DG], F32, tag="gsb", name="gsb")
        nc.sync.dma_start(
            out=gsb[:], in_=cc_out[:].rearrange("(j p) s -> p j s", p=P))
        a0b0 = pC.tile([P, 2 * DG], F32, tag="a0b0", name="a0b0")
        nc.vector.memset(a0b0[:, 0:DG], 0.0)
        nc.vector.memset(a0b0[:, DG:2 * DG], DEN_EPS)
        for j in range(n_cores):
            nc.vector.scalar_tensor_tensor(
                a0b0[:], gsb[:, j, :], selt[:, j:j + 1], a0b0[:],
                ALU.mult, ALU.add)

        # ============ Phase D: WKV scans + rwkv ============
        pRw = tc.alloc_tile_pool(name="pRw", bufs=1)
        rwkv = pRw.tile([P, DG, RS], BF16, tag="rwkv")
        pD = tc.alloc_tile_pool(name="pD", bufs=2)
        for g in range(DG):
            ekg = pD.tile([P, RS], BF16, tag="ekg", name="ekg")
            nc.sync.dma_start(out=ekg[:], in_=ekdv[:, g, :])
            xkg = pD.tile([P, RS], BF16, tag="xkg", name="xkg")
            nc.sync.dma_start(out=xkg[:], in_=xkvdv[:, g, :])
            srg = pD.tile([P, RS], BF16, tag="srg", name="srg")
            nc.sync.dma_start(out=srg[:], in_=srdv[:, g, :])
            ewbd = ccol(g, I_EW).to_broadcast([P, RS])
            abuf = pD.tile([P, RS + 1], F32, tag="abuf", name="abuf")
            nc.gpsimd.tensor_copy(out=abuf[:, 0:1], in_=a0b0[:, g:g + 1])
            nc.vector.tensor_tensor_scan(
                abuf[:, 1:RS + 1], ewbd, xkg[:], a0b0[:, g:g + 1],
                ALU.mult, ALU.add)
            bbuf = pD.tile([P, RS + 1], F32, tag="bbuf", name="bbuf")
            nc.gpsimd.tensor_copy(out=bbuf[:, 0:1],
                                  in_=a0b0[:, DG + g:DG + g + 1])
            nc.vector.tensor_tensor_scan(
                bbuf[:, 1:RS + 1], ewbd, ekg[:],
                a0b0[:, DG + g:DG + g + 1], ALU.mult, ALU.add)
            num = pD.tile([P, RS], F32, tag="num", name="num")
            nc.vector.scalar_tensor_tensor(
                num[:], xkg[:], ccol(g, I_EU), abuf[:, 0:RS],
                ALU.mult, ALU.add)
            den = pD.tile([P, RS], F32, tag="den", name="den")
            nc.vector.scalar_tensor_tensor(
                den[:], ekg[:], ccol(g, I_EU), bbuf[:, 0:RS],
                ALU.mult, ALU.add)
            rden = pD.tile([P, RS], F32, tag="rden", name="rden")
            nc.vector.reciprocal_approx_fast(out=rden[:], in_=den[:])
            nc.gpsimd.tensor_tensor(num[:], num[:], rden[:], ALU.mult)
            nc.gpsimd.tensor_tensor(rwkv[:, g, :], num[:], srg[:], ALU.mult)
        pD.release()

        # ============ Phase E: Wo matmul -> x2 (to DRAM) ============
        wpE = tc.alloc_tile_pool(name="wpE", bufs=2, side="right")
        spE = tc.alloc_tile_pool(name="spE", bufs=3, side="right")
        psE = tc.alloc_tile_pool(name="psE", bufs=2, space="PSUM")

        CBLK = min(512, Cc)
        for c0, csz in _splits(Cc, CBLK):
            wbuf = wpE.tile([P, DG, CBLK], mm_dt, tag="wo", name="wo")
            nc.sync.dma_start(out=wbuf[:, :, :csz], in_=wo[:, :, c0:c0 + csz])
            for gl in range(csz // P):
                g_c = (c0 + gl * P) // P
                for t0, tsz in _even_splits(RS, TS):
                    ps = psE.tile([P, TS], F32, tag="wo_ps", name="wo_ps")
                    for gi in range(DG):
                        nc.tensor.matmul(
                            ps[:, :tsz], wbuf[:, gi, gl * P:(gl + 1) * P],
                            rwkv[:, gi, t0:t0 + tsz],
                            start=(gi == 0), stop=(gi == DG - 1))
                    xst = spE.tile([P, TS], F32, tag="xst", name="xst")
                    nc.sync.dma_start(
                        out=xst[:, :tsz],
                        in_=xTv[:, g_c, 1 + t0:1 + t0 + tsz])
                    x2st = spE.tile([P, TS], F32, tag="x2st", name="x2st")
                    nc.vector.tensor_tensor(x2st[:, :tsz], xst[:, :tsz],
                                            ps[:, :tsz], ALU.add)
                    nc.sync.dma_start(out=x2dv[:, g_c, t0:t0 + tsz],
                                      in_=x2st[:, :tsz])
        psE.release()
        spE.release()
        wpE.release()
        pC.release()
        pRw.release()

        # ============ Phase F: LN2 + mixes2 ============
        pMx2 = tc.alloc_tile_pool(name="pMx2", bufs=1)
        pXr2 = tc.alloc_tile_pool(name="pXr2", bufs=1)
        pG2 = tc.alloc_tile_pool(name="pG2", bufs=1)
        xk2 = pMx2.tile([P, CG, RO], BF16, tag="xk2")
        xr2 = pXr2.tile([P, CG, RO], BF16, tag="xr2")
        g2 = pG2.tile([P, CG, RS], BF16, tag="g2")
        ln_stream(x2dv, RS, I_LN2W, g2, "ln2")
        nc.vector.tensor_scalar_mul(g2[:, :, 0:1], g2[:, :, 0:1], m0[:])

        spF = tc.alloc_tile_pool(name="spF", bufs=2)
        for g in range(CG):
            dmix = spF.tile([P, RO], BF16, tag="dmix2", name="dmix2")
            nc.vector.tensor_tensor(dmix[:], g2[:, g, 1:RS], g2[:, g, 0:RO],
                                    ALU.subtract)
            nc.vector.scalar_tensor_tensor(
                xk2[:, g, :], dmix[:], ccol(g, I_CMK), g2[:, g, 0:RO],
                ALU.mult, ALU.add)
            nc.vector.scalar_tensor_tensor(
                xr2[:, g, :], dmix[:], ccol(g, I_CMR), g2[:, g, 0:RO],
                ALU.mult, ALU.add)
        spF.release()
        pG2.release()

        # ============ Phase G: r2 = sigmoid(xr2 @ WcrT) -> DRAM ============
        wpG = tc.alloc_tile_pool(name="wpG", bufs=2)
        spG = tc.alloc_tile_pool(name="spG", bufs=2)
        psG = tc.alloc_tile_pool(name="psG", bufs=2, space="PSUM")
        for c0, csz in _splits(Cc, CBLK):
            wbuf = wpG.tile([P, CG, CBLK], mm_dt, tag="wcr", name="wcr")
            nc.sync.dma_start(out=wbuf[:, :, :csz], in_=wcr[:, :, c0:c0 + csz])
            for gl in range(csz // P):
                g_c = (c0 + gl * P) // P
                for t0, tsz in _splits(RO, TS):
                    ps = psG.tile([P, TS], F32, tag="wcr_ps", name="wcr_ps")
                    for gi in range(CG):
                        nc.tensor.matmul(
                            ps[:, :tsz], wbuf[:, gi, gl * P:(gl + 1) * P],
                            xr2[:, gi, t0:t0 + tsz],
                            start=(gi == 0), stop=(gi == CG - 1))
                    sgt = spG.tile([P, TS], BF16, tag="sgt", name="sgt")
                    nc.scalar.activation(sgt[:, :tsz], ps[:, :tsz],
                                         ACT.Sigmoid)
                    nc.sync.dma_start(out=sgdv[:, g_c, t0:t0 + tsz],
                                      in_=sgt[:, :tsz])
        psG.release()
        spG.release()
        wpG.release()
        pXr2.release()

        # ============ Phase H: FFN ============
        FBLK = min(512, Ff)
        FQ = 16 if FG >= 16 else FG
        for t0, tsz in _splits(RO, TS):
            pH = tc.alloc_tile_pool(name=f"pH{t0}", bufs=1)
            wpH = tc.alloc_tile_pool(name=f"wpH{t0}", bufs=2)
            psH = tc.alloc_tile_pool(name=f"psH{t0}", bufs=2, space="PSUM")
            psKV = tc.alloc_tile_pool(name=f"psKV{t0}", bufs=1, space="PSUM")
            kfsq = pH.tile([P, FG, TS], BF16, tag="kfsq", name="kfsq")
            # FFN1: kf = relu(xk2 @ WckT)^2
            for f0, fsz in _splits(Ff, FBLK):
                wbuf = wpH.tile([P, CG, FBLK], mm_dt, tag="wf", name="wf")
                nc.sync.dma_start(out=wbuf[:, :, :fsz],
                                  in_=wck[:, :, f0:f0 + fsz])
                for fl in range(fsz // P):
                    g_f = (f0 + fl * P) // P
                    ps = psH.tile([P, TS], F32, tag="ffn1_ps", name="ffn1_ps")
                    for gi in range(CG):
                        nc.tensor.matmul(
                            ps[:, :tsz], wbuf[:, gi, fl * P:(fl + 1) * P],
                            xk2[:, gi, t0:t0 + tsz],
                            start=(gi == 0), stop=(gi == CG - 1))
                    nc.scalar.activation(kfsq[:, g_f, :tsz], ps[:, :tsz],
                                         ACT.Relu)
                    nc.vector.tensor_tensor(kfsq[:, g_f, :tsz],
                                            kfsq[:, g_f, :tsz],
                                            kfsq[:, g_f, :tsz], ALU.mult)
            # FFN2 + final: out = x2 + sg * (kfsq @ WcvT)
            for c0, csz in _splits(Cc, CBLK):
                kvps = [psKV.tile([P, TS], F32, tag=f"kv_ps{i}",
                                  name=f"kv_ps{i}")
                        for i in range(csz // P)]
                nq = (FG + FQ - 1) // FQ
                for q in range(nq):
                    f_lo = q * FQ
                    f_n = min(FQ, FG - f_lo)
                    wbuf = wpH.tile([P, FQ, CBLK], mm_dt, tag="wf2",
                                    name="wf2")
                    nc.sync.dma_start(
                        out=wbuf[:, :f_n, :csz],
                        in_=wcv[:, f_lo:f_lo + f_n, c0:c0 + csz])
                    for gl in range(csz // P):
                        for fi in range(f_n):
                            nc.tensor.matmul(
                                kvps[gl][:, :tsz],
                                wbuf[:, fi, gl * P:(gl + 1) * P],
                                kfsq[:, f_lo + fi, :tsz],
                                start=(q == 0 and fi == 0),
                                stop=(q == nq - 1 and fi == f_n - 1))
                for gl in range(csz // P):
                    g_c = (c0 + gl * P) // P
                    x2s = wpH.tile([P, TS], F32, tag="x2s", name="x2s")
                    nc.sync.dma_start(
                        out=x2s[:, :tsz],
                        in_=x2dv[:, g_c, 1 + t0:1 + t0 + tsz])
                    sgs = wpH.tile([P, TS], BF16, tag="sgs", name="sgs")
                    nc.sync.dma_start(out=sgs[:, :tsz],
                                      in_=sgdv[:, g_c, t0:t0 + tsz])
                    ot = wpH.tile([P, TS], F32, tag="ot", name="ot")
                    nc.vector.tensor_tensor(ot[:, :tsz], sgs[:, :tsz],
                                            kvps[gl][:, :tsz], ALU.mult)
                    nc.vector.tensor_tensor(ot[:, :tsz], ot[:, :tsz],
                                            x2s[:, :tsz], ALU.add)
                    nc.sync.dma_start(out=outTv[:, g_c, t0:t0 + tsz],
                                      in_=ot[:, :tsz])
            for p in (psKV, psH, wpH, pH):
                p.release()
        pMx2.release()
        dram.release()
        const.release()

    nc.compile()
    return nc


_PROGRAM_CACHE = {}


def _get_program(key, **kw):
    if key not in _PROGRAM_CACHE:
        _PROGRAM_CACHE[key] = build_program(**kw)
    return _PROGRAM_CACHE[key]


def _host_prep(inputs, Cc=C, Dd=D_ATT, Ff=D_FFN, Bb=B, Tt=T, n_cores=N_CORES):
    """Build per-core input maps (numpy only)."""
    P = 128
    CG, DG, FG = Cc // P, Dd // P, Ff // P
    half = Tt // 2
    RO, RS, R = half, half + 1, half + 2
    bf = ml_dtypes.bfloat16

    f = {k: np.asarray(v, np.float32) for k, v in inputs.items()}
    x = f["x"]

    def swz(wT, kg):  # [K, N] -> [128, kg, N] with [p, gi, n] = wT[gi*128+p, n]
        Kdim, Ndim = wT.shape
        return np.ascontiguousarray(
            wT.reshape(kg, P, Ndim).transpose(1, 0, 2)).astype(bf)

    wk_h = swz(f["Wk"].T, CG)
    wv_h = swz(f["Wv"].T, CG)
    wr_h = swz(f["Wr"].T, CG)
    wo_h = swz(f["Wo"].T, DG)
    wck_h = swz(f["Wck"].T, CG)
    wcv_h = swz(f["Wcv"].T, FG)
    wcr_h = swz(f["Wcr"].T, CG)

    def col(v):  # [C] -> [128, CG]
        return np.ascontiguousarray(
            np.asarray(v, np.float32).reshape(-1).reshape(CG, P).T)

    ew = np.exp(-np.exp(f["time_decay"].astype(np.float64)))
    cvec_h = np.stack([
        col(f["ln1_w"]), col(f["ln1_b"]),
        col(f["tm_k"]), col(f["tm_v"]), col(f["tm_r"]),
        col(ew.astype(np.float32)), col(np.exp(f["time_first"])),
        col(f["ln2_w"]), col(f["ln2_b"]),
        col(f["cm_k"]), col(f["cm_r"]),
    ], axis=-1).astype(np.float32)  # [128, CG, 11]

    in_maps = []
    for core in range(n_cores):
        b, hh = core // 2, core % 2
        t0 = hh * half
        xr = np.zeros((R, Cc), np.float32)
        lo = t0 - 2
        src_lo = max(lo, 0)
        xr[src_lo - lo:, :] = x[b, src_lo:t0 + RO, :]
        m0 = np.full((P, 1), float(hh), np.float32)
        sel = np.zeros((P, n_cores), np.float32)
        if hh == 1:
            sel[:, core - 1] = 1.0
        in_maps.append({
            "xT": np.ascontiguousarray(xr.T),
            "wk": wk_h, "wv": wv_h, "wr": wr_h, "wo": wo_h,
            "wck": wck_h, "wcv": wcv_h, "wcr": wcr_h,
            "cvec": cvec_h, "m0": m0, "sel": sel,
        })
    return in_maps


def kernel(**inputs):
    in_maps = _host_prep(inputs)
    nc = _get_program("full")
    res = run_bass_kernel_spmd(nc, in_maps, core_ids=list(range(N_CORES)))
    half = T // 2
    out = np.empty((B, T, C), np.float32)
    for core in range(N_CORES):
        b, hh = core // 2, core % 2
        out[b, hh * half:(hh + 1) * half, :] = res.results[core]["outT"].T
    return out


# revision 15
# speedup vs baseline: 1.0012x; 1.0012x over previous
"""RWKV-4 block (TimeMix WKV recurrence + ChannelMix) on 8 Trainium2 NeuronCores.

Sharding: 8 cores = 4 batch elements x 2 halves of T (1024 output rows each).
All compute is row-local except the WKV scan carry state, which is passed
between half-pairs with one tiny AllGather ([128, 2*DG] fp32 per core).

Device layout: channels-on-partitions [c, t].  The host pre-transposes x and
all weight matrices so every DMA is contiguous.  WKV runs as a hardware
tensor_tensor_scan (state = ew*state + x) per 128-channel group.  Large
intermediates (ek, ek*v, sigmoid(r), x2, sigmoid(r2)) are spilled to DRAM and
streamed back so SBUF tile-pool lifetimes nest (stack allocator).
"""

import os
import sys

import numpy as np

for _p in ("/opt/trn_rl_repo", "/root/.axon_site/_ro/trn_rl_repo"):
    if os.path.isdir(_p) and _p not in sys.path:
        sys.path.insert(0, _p)

import ml_dtypes  # noqa: E402

import concourse.bass as bass  # noqa: E402,F401
import concourse.mybir as mybir  # noqa: E402
import concourse.tile as tile  # noqa: E402
from concourse import bacc  # noqa: E402
from concourse.bass_utils import run_bass_kernel_spmd  # noqa: E402

F32 = mybir.dt.float32
F32R = mybir.dt.float32r
BF16 = mybir.dt.bfloat16
ALU = mybir.AluOpType
ACT = mybir.ActivationFunctionType

B, T, C, D_ATT, D_FFN = 4, 2048, 2048, 2048, 8192
EPS = 1e-5
N_CORES = 8
DEN_EPS = 1e-30  # keeps Den finite on the virtual row of first-half cores


def _splits(total, sz):
    return [(s, min(sz, total - s)) for s in range(0, total, sz)]


def _even_splits(total, mx):
    n = -(-total // mx)
    base, rem = divmod(total, n)
    out, s = [], 0
    for i in range(n):
        sz = base + (1 if i < rem else 0)
        out.append((s, sz))
        s += sz
    return out


def build_program(Cc=C, Dd=D_ATT, Ff=D_FFN, rows_out=T // 2, n_cores=N_CORES,
                  mm_dt=BF16, no_collective=False):
    """Build the (uniform SPMD) Bass program."""
    P = 128
    CG, DG, FG = Cc // P, Dd // P, Ff // P
    RO = rows_out              # output rows per core
    RS = RO + 1                # scan rows (one leading row)
    R = RS + 1                 # loaded x rows (two leading rows)
    NV = 11

    nc = bacc.Bacc("TRN2", target_bir_lowering=False, debug=False,
                   num_devices=n_cores)

    xT = nc.dram_tensor("xT", [Cc, R], F32, kind="ExternalInput").ap()
    wk = nc.dram_tensor("wk", [P, CG, Dd], mm_dt, kind="ExternalInput").ap()
    wv = nc.dram_tensor("wv", [P, CG, Dd], mm_dt, kind="ExternalInput").ap()
    wr = nc.dram_tensor("wr", [P, CG, Dd], mm_dt, kind="ExternalInput").ap()
    wo = nc.dram_tensor("wo", [P, DG, Cc], mm_dt, kind="ExternalInput").ap()
    wck = nc.dram_tensor("wck", [P, CG, Ff], mm_dt, kind="ExternalInput").ap()
    wcv = nc.dram_tensor("wcv", [P, FG, Cc], mm_dt, kind="ExternalInput").ap()
    wcr = nc.dram_tensor("wcr", [P, CG, Cc], mm_dt, kind="ExternalInput").ap()
    cvec = nc.dram_tensor("cvec", [P, CG, NV], F32, kind="ExternalInput").ap()
    m0d = nc.dram_tensor("m0", [P, 1], F32, kind="ExternalInput").ap()
    seld = nc.dram_tensor("sel", [P, n_cores], F32, kind="ExternalInput").ap()
    outT = nc.dram_tensor("outT", [Cc, RO], F32, kind="ExternalOutput").ap()

    xTv = xT.rearrange("(g p) r -> p g r", p=P)
    outTv = outT.rearrange("(g p) r -> p g r", p=P)

    I_LN1W, I_LN1B, I_TMK, I_TMV, I_TMR, I_EW, I_EU, I_LN2W, I_LN2B, \
        I_CMK, I_CMR = range(NV)

    TS = 512                 # matmul moving free-dim tile
    LTS = min(256, RS)       # layernorm streaming tile

    with tile.TileContext(nc) as tc:
        const = tc.alloc_tile_pool(name="const", bufs=1)
        con = const.tile([P, CG, NV], F32, tag="con")
        nc.sync.dma_start(out=con[:], in_=cvec)
        m0 = const.tile([P, 1], F32, tag="m0")
        nc.sync.dma_start(out=m0[:], in_=m0d)
        selt = const.tile([P, n_cores], F32, tag="sel")
        nc.sync.dma_start(out=selt[:], in_=seld)
        onesc = const.tile([P, 1], F32, tag="ones")
        nc.vector.memset(onesc[:], 1.0)
        onesb = const.tile([P, 1], BF16, tag="onesb")
        nc.vector.memset(onesb[:], 1.0)
        epsc = const.tile([1, 1], F32, tag="epsc")
        nc.vector.memset(epsc[:], EPS)
        onesP = const.tile([1, P], F32, tag="onesP")
        nc.vector.memset(onesP[:], 1.0)

        def ccol(g, i):
            return con[:, g, i:i + 1]

        dram = tc.alloc_tile_pool(name="dram", bufs=1, space="DRAM")
        ekdram = dram.tile([Dd, RS], BF16)
        ekdv = ekdram.rearrange("(g p) r -> p g r", p=P)
        xkvdram = dram.tile([Dd, RS], BF16)
        xkvdv = xkvdram.rearrange("(g p) r -> p g r", p=P)
        srdram = dram.tile([Dd, RS], BF16)
        srdv = srdram.rearrange("(g p) r -> p g r", p=P)
        x2dram = dram.tile([Cc, RS], F32)
        x2dv = x2dram.rearrange("(g p) r -> p g r", p=P)
        sgdram = dram.tile([Cc, RO], BF16)
        sgdv = sgdram.rearrange("(g p) r -> p g r", p=P)
        cc_in = dram.tile([P, 2 * DG], F32)
        cc_out = dram.tile([P * n_cores, 2 * DG], F32)

        # ---- LayerNorm over partition-dim channels, streaming from DRAM ----
        def ln_stream(src_v, nrows, iw, out_sb, name):
            """src_v: DRAM view [P, CG, nrows] fp32 -> out_sb [P,CG,nrows] bf16."""
            st = tc.alloc_tile_pool(name=f"{name}_st", bufs=1)
            sp = tc.alloc_tile_pool(name=f"{name}_sp", bufs=2)
            psum = tc.alloc_tile_pool(name=f"{name}_ps", bufs=2, space="PSUM")
            ssum = st.tile([1, nrows], F32, tag="sum", name="ssum")
            ssq = st.tile([1, nrows], F32, tag="sq", name="ssq")
            for t0, tsz in _splits(nrows, LTS):
                xls = sp.tile([P, CG, LTS], F32, tag="xls", name="xls")
                nc.sync.dma_start(out=xls[:, :, :tsz],
                                  in_=src_v[:, :, t0:t0 + tsz])
                xsq = sp.tile([P, CG, LTS], BF16, tag="lnsq", name="xsq")
                nc.scalar.activation(xsq[:, :, :tsz], xls[:, :, :tsz],
                                     ACT.Square)
                xbf = sp.tile([P, CG, LTS], BF16, tag="lnbf", name="xbf")
                nc.vector.tensor_copy(out=xbf[:, :, :tsz],
                                      in_=xls[:, :, :tsz])
                ps = psum.tile([1, LTS], F32, tag="ln_ps", name="ps")
                ps2 = psum.tile([1, LTS], F32, tag="ln_ps2", name="ps2")
                for g in range(CG):
                    nc.tensor.matmul(
                        ps[:, :tsz], onesb[:], xbf[:, g, :tsz],
                        start=(g == 0), stop=(g == CG - 1))
                    nc.tensor.matmul(
                        ps2[:, :tsz], onesb[:], xsq[:, g, :tsz],
                        start=(g == 0), stop=(g == CG - 1))
                nc.vector.tensor_copy(out=ssum[:, t0:t0 + tsz],
                                      in_=ps[:, :tsz])
                nc.vector.tensor_copy(out=ssq[:, t0:t0 + tsz],
                                      in_=ps2[:, :tsz])
            mu = st.tile([1, nrows], F32, tag="mu", name="mu")
            rstd = st.tile([1, nrows], F32, tag="rstd", name="rstd")
            var = st.tile([1, nrows], F32, tag="var", name="var")
            musq = st.tile([1, nrows], F32, tag="musq", name="musq")
            nc.vector.tensor_scalar_mul(mu[:], ssum[:], 1.0 / Cc)
            nc.vector.tensor_scalar_mul(var[:], ssq[:], 1.0 / Cc)
            nc.vector.tensor_tensor(musq[:], mu[:], mu[:], ALU.mult)
            nc.vector.tensor_tensor(var[:], var[:], musq[:], ALU.subtract)
            nc.scalar.activation(var[:], var[:], ACT.Ln, bias=epsc[:])
            nc.scalar.activation(rstd[:], var[:], ACT.Exp, scale=-0.5)
            for t0, tsz in _splits(nrows, LTS):
                xls = sp.tile([P, CG, LTS], F32, tag="xls", name="xls")
                nc.sync.dma_start(out=xls[:, :, :tsz],
                                  in_=src_v[:, :, t0:t0 + tsz])
                # broadcast per-row stats to all 128 partitions via K=1 matmul
                mups = psum.tile([P, LTS], F32, tag="mups", name="mups")
                nc.tensor.matmul(mups[:, :tsz], onesP[:],
                                 mu[:, t0:t0 + tsz],
                                 start=True, stop=True)
                rsps = psum.tile([P, LTS], F32, tag="rsps", name="rsps")
                nc.tensor.matmul(rsps[:, :tsz], onesP[:],
                                 rstd[:, t0:t0 + tsz],
                                 start=True, stop=True)
                for g in range(CG):
                    xm = sp.tile([P, LTS], F32, tag="ln_xm", name="xm")
                    nc.vector.tensor_tensor(xm[:, :tsz], xls[:, g, :tsz],
                                            mups[:, :tsz], ALU.subtract)
                    nc.vector.scalar_tensor_tensor(
                        out_sb[:, g, t0:t0 + tsz], xm[:, :tsz], ccol(g, iw),
                        rsps[:, :tsz], ALU.mult, ALU.mult)
            for p in (psum, sp, st):
                p.release()

        # ================= Phase A: LN1 =================
        pHs = tc.alloc_tile_pool(name="pHs", bufs=1)
        hs = pHs.tile([P, CG, R], BF16, tag="hs")
        ln_stream(xTv, R, I_LN1W, hs, "ln1")
        # zero the two lead rows on first-half cores (time_shift zero pad)
        nc.vector.tensor_scalar_mul(hs[:, :, 0:2], hs[:, :, 0:2], m0[:])

        # ============ Phase B: mixes + k/v/r matmuls ============
        pMix = tc.alloc_tile_pool(name="pMix", bufs=2)
        wpB = tc.alloc_tile_pool(name="wpB", bufs=2)
        stg = tc.alloc_tile_pool(name="stg", bufs=3)
        psB = tc.alloc_tile_pool(name="psB", bufs=4, space="PSUM")

        DBLK = min(512, Dd)

        def make_mix(icoef):
            mix = pMix.tile([P, CG, RS], BF16, tag="mix", name="mix")
            for g in range(CG):
                dmix = stg.tile([P, RS], BF16, tag="dmix", name="dmix")
                nc.vector.tensor_tensor(dmix[:], hs[:, g, 1:R],
                                        hs[:, g, 0:RS], ALU.subtract)
                nc.vector.scalar_tensor_tensor(
                    mix[:, g, :], dmix[:], ccol(g, icoef), hs[:, g, 0:RS],
                    ALU.mult, ALU.add)
            return mix

        def mm_phase(wdram, rhs, n_out, nrows, evict):
            for d0, dsz in _splits(n_out, DBLK):
                wbuf = wpB.tile([P, CG, DBLK], mm_dt, tag="w3", name="w3")
                nc.sync.dma_start(out=wbuf[:, :, :dsz],
                                  in_=wdram[:, :, d0:d0 + dsz])
                for gl in range(dsz // P):
                    g_out = (d0 + gl * P) // P
                    for t0, tsz in _even_splits(nrows, TS):
                        ps = psB.tile([P, TS], F32, tag="mm_ps", name="mm_ps")
                        for gi in range(CG):
                            nc.tensor.matmul(
                                ps[:, :tsz],
                                wbuf[:, gi, gl * P:(gl + 1) * P],
                                rhs[:, gi, t0:t0 + tsz],
                                start=(gi == 0), stop=(gi == CG - 1))
                        evict(g_out, t0, tsz, ps)

        def evict_k(g, t0, tsz, ps):
            est = stg.tile([P, TS], BF16, tag="est", name="est")
            nc.scalar.activation(est[:, :tsz], ps[:, :tsz], ACT.Exp)
            if t0 == 0:  # mask the virtual lead row on first-half cores
                nc.vector.tensor_scalar_mul(est[:, 0:1], est[:, 0:1], m0[:])
            nc.sync.dma_start(out=ekdv[:, g, t0:t0 + tsz], in_=est[:, :tsz])

        def evict_v(g, t0, tsz, ps):
            eld = stg.tile([P, TS], BF16, tag="eld", name="eld")
            nc.sync.dma_start(out=eld[:, :tsz], in_=ekdv[:, g, t0:t0 + tsz])
            xst = stg.tile([P, TS], BF16, tag="xst", name="xst")
            nc.vector.tensor_tensor(xst[:, :tsz], eld[:, :tsz], ps[:, :tsz],
                                    ALU.mult)
            nc.sync.dma_start(out=xkvdv[:, g, t0:t0 + tsz], in_=xst[:, :tsz])

        def evict_r(g, t0, tsz, ps):
            srt = stg.tile([P, TS], BF16, tag="srt", name="srt")
            nc.scalar.activation(srt[:, :tsz], ps[:, :tsz], ACT.Sigmoid)
            nc.sync.dma_start(out=srdv[:, g, t0:t0 + tsz], in_=srt[:, :tsz])

        mixk = make_mix(I_TMK)
        mm_phase(wk, mixk, Dd, RS, evict_k)
        mixv = make_mix(I_TMV)
        mm_phase(wv, mixv, Dd, RS, evict_v)
        mixr = make_mix(I_TMR)
        mm_phase(wr, mixr, Dd, RS, evict_r)

        psB.release()
        stg.release()
        wpB.release()
        pMix.release()
        pHs.release()

        # ============ Phase C: boundary states + AllGather ============
        # Right-side pool: C's DVE scans overlap phase B's matmuls without
        # waiting on B's pool-zone releases.
        pC = tc.alloc_tile_pool(name="pC", bufs=2, side="right")

        state = pC.tile([P, 2 * DG], F32, tag="state", name="state")
        for g in range(DG):
            ekg = pC.tile([P, RS], BF16, tag="ekg", name="ekg")
            nc.sync.dma_start(out=ekg[:], in_=ekdv[:, g, :])
            xkg = pC.tile([P, RS], BF16, tag="xkg", name="xkg")
            nc.sync.dma_start(out=xkg[:], in_=xkvdv[:, g, :])
            ewbc = ccol(g, I_EW).to_broadcast([P, RS - 1])
            apre = pC.tile([P, RS - 1], F32, tag="apre", name="apre")
            nc.vector.tensor_tensor_scan(
                apre[:], ewbc, xkg[:, :RS - 1], 0.0, ALU.mult, ALU.add)
            nc.gpsimd.tensor_copy(out=state[:, g:g + 1],
                                  in_=apre[:, RS - 2:RS - 1])
            bpre = pC.tile([P, RS - 1], F32, tag="bpre", name="bpre")
            nc.vector.tensor_tensor_scan(
                bpre[:], ewbc, ekg[:, :RS - 1], 0.0, ALU.mult, ALU.add)
            nc.gpsimd.tensor_copy(out=state[:, DG + g:DG + g + 1],
                                  in_=bpre[:, RS - 2:RS - 1])
        nc.sync.dma_start(out=cc_in[:], in_=state[:])
        if not no_collective:
            nc.gpsimd.collective_compute(
                "AllGather", ALU.bypass,
                replica_groups=[list(range(n_cores))],
                ins=[cc_in[:].opt()], outs=[cc_out[:].opt()])
        else:  # timing-equivalent stand-in for TimelineSim profiling
            for jj in range(n_cores):
                nc.sync.dma_start(out=cc_out[jj * P:(jj + 1) * P, :],
                                  in_=cc_in[:])
        gsb = pC.tile([P, n_cores, 2 * DG], F32, tag="gsb", name="gsb")
        nc.sync.dma_start(
            out=gsb[:], in_=cc_out[:].rearrange("(j p) s -> p j s", p=P))
        a0b0 = pC.tile([P, 2 * DG], F32, tag="a0b0", name="a0b0")
        nc.vector.memset(a0b0[:, 0:DG], 0.0)
        nc.vector.memset(a0b0[:, DG:2 * DG], DEN_EPS)
        for j in range(n_cores):
            nc.vector.scalar_tensor_tensor(
                a0b0[:], gsb[:, j, :], selt[:, j:j + 1], a0b0[:],
                ALU.mult, ALU.add)

        # ============ Phase D: WKV scans + rwkv ============
        pRw = tc.alloc_tile_pool(name="pRw", bufs=1)
        rwkv = pRw.tile([P, DG, RS], BF16, tag="rwkv")
        pD = tc.alloc_tile_pool(name="pD", bufs=2)
        for g in range(DG):
            ekg = pD.tile([P, RS], BF16, tag="ekg", name="ekg")
            nc.sync.dma_start(out=ekg[:], in_=ekdv[:, g, :])
            xkg = pD.tile([P, RS], BF16, tag="xkg", name="xkg")
            nc.sync.dma_start(out=xkg[:], in_=xkvdv[:, g, :])
            srg = pD.tile([P, RS], BF16, tag="srg", name="srg")
            nc.sync.dma_start(out=srg[:], in_=srdv[:, g, :])
            ewbd = ccol(g, I_EW).to_broadcast([P, RS])
            abuf = pD.tile([P, RS + 1], F32, tag="abuf", name="abuf")
            nc.gpsimd.tensor_copy(out=abuf[:, 0:1], in_=a0b0[:, g:g + 1])
            nc.vector.tensor_tensor_scan(
                abuf[:, 1:RS + 1], ewbd, xkg[:], a0b0[:, g:g + 1],
                ALU.mult, ALU.add)
            bbuf = pD.tile([P, RS + 1], F32, tag="bbuf", name="bbuf")
            nc.gpsimd.tensor_copy(out=bbuf[:, 0:1],
                                  in_=a0b0[:, DG + g:DG + g + 1])
            nc.vector.tensor_tensor_scan(
                bbuf[:, 1:RS + 1], ewbd, ekg[:],
                a0b0[:, DG + g:DG + g + 1], ALU.mult, ALU.add)
            num = pD.tile([P, RS], F32, tag="num", name="num")
            nc.vector.scalar_tensor_tensor(
                num[:], xkg[:], ccol(g, I_EU), abuf[:, 0:RS],
                ALU.mult, ALU.add)
            den = pD.tile([P, RS], F32, tag="den", name="den")
            nc.vector.scalar_tensor_tensor(
                den[:], ekg[:], ccol(g, I_EU), bbuf[:, 0:RS],
                ALU.mult, ALU.add)
            rden = pD.tile([P, RS], F32, tag="rden", name="rden")
            nc.vector.reciprocal_approx_fast(out=rden[:], in_=den[:])
            nc.gpsimd.tensor_tensor(num[:], num[:], rden[:], ALU.mult)
            nc.gpsimd.tensor_tensor(rwkv[:, g, :], num[:], srg[:], ALU.mult)
        pD.release()

        # ============ Phase E: Wo matmul -> x2 (to DRAM) ============
        wpE = tc.alloc_tile_pool(name="wpE", bufs=2, side="right")
        spE = tc.alloc_tile_pool(name="spE", bufs=3, side="right")
        psE = tc.alloc_tile_pool(name="psE", bufs=2, space="PSUM")

        CBLK = min(512, Cc)
        for c0, csz in _splits(Cc, CBLK):
            wbuf = wpE.tile([P, DG, CBLK], mm_dt, tag="wo", name="wo")
            nc.sync.dma_start(out=wbuf[:, :, :csz], in_=wo[:, :, c0:c0 + csz])
            for gl in range(csz // P):
                g_c = (c0 + gl * P) // P
                for t0, tsz in _even_splits(RS, TS):
                    ps = psE.tile([P, TS], F32, tag="wo_ps", name="wo_ps")
                    for gi in range(DG):
                        nc.tensor.matmul(
                            ps[:, :tsz], wbuf[:, gi, gl * P:(gl + 1) * P],
                            rwkv[:, gi, t0:t0 + tsz],
                            start=(gi == 0), stop=(gi == DG - 1))
                    xst = spE.tile([P, TS], F32, tag="xst", name="xst")
                    nc.sync.dma_start(
                        out=xst[:, :tsz],
                        in_=xTv[:, g_c, 1 + t0:1 + t0 + tsz])
                    x2st = spE.tile([P, TS], F32, tag="x2st", name="x2st")
                    nc.vector.tensor_tensor(x2st[:, :tsz], xst[:, :tsz],
                                            ps[:, :tsz], ALU.add)
                    nc.sync.dma_start(out=x2dv[:, g_c, t0:t0 + tsz],
                                      in_=x2st[:, :tsz])
        psE.release()
        spE.release()
        wpE.release()
        pC.release()
        pRw.release()

        # ============ Phase F: LN2 + mixes2 ============
        pMx2 = tc.alloc_tile_pool(name="pMx2", bufs=1)
        pXr2 = tc.alloc_tile_pool(name="pXr2", bufs=1)
        pG2 = tc.alloc_tile_pool(name="pG2", bufs=1)
        xk2 = pMx2.tile([P, CG, RO], BF16, tag="xk2")
        xr2 = pXr2.tile([P, CG, RO], BF16, tag="xr2")
        g2 = pG2.tile([P, CG, RS], BF16, tag="g2")
        ln_stream(x2dv, RS, I_LN2W, g2, "ln2")
        nc.vector.tensor_scalar_mul(g2[:, :, 0:1], g2[:, :, 0:1], m0[:])

        spF = tc.alloc_tile_pool(name="spF", bufs=2)
        for g in range(CG):
            dmix = spF.tile([P, RO], BF16, tag="dmix2", name="dmix2")
            nc.vector.tensor_tensor(dmix[:], g2[:, g, 1:RS], g2[:, g, 0:RO],
                                    ALU.subtract)
            nc.vector.scalar_tensor_tensor(
                xk2[:, g, :], dmix[:], ccol(g, I_CMK), g2[:, g, 0:RO],
                ALU.mult, ALU.add)
            nc.vector.scalar_tensor_tensor(
                xr2[:, g, :], dmix[:], ccol(g, I_CMR), g2[:, g, 0:RO],
                ALU.mult, ALU.add)
        spF.release()
        pG2.release()

        # ============ Phase G: r2 = sigmoid(xr2 @ WcrT) -> DRAM ============
        wpG = tc.alloc_tile_pool(name="wpG", bufs=2)
        spG = tc.alloc_tile_pool(name="spG", bufs=2)
        psG = tc.alloc_tile_pool(name="psG", bufs=3, space="PSUM")
        for c0, csz in _splits(Cc, CBLK):
            wbuf = wpG.tile([P, CG, CBLK], mm_dt, tag="wcr", name="wcr")
            nc.sync.dma_start(out=wbuf[:, :, :csz], in_=wcr[:, :, c0:c0 + csz])
            for gl in range(csz // P):
                g_c = (c0 + gl * P) // P
                for t0, tsz in _splits(RO, TS):
                    ps = psG.tile([P, TS], F32, tag="wcr_ps", name="wcr_ps")
                    for gi in range(CG):
                        nc.tensor.matmul(
                            ps[:, :tsz], wbuf[:, gi, gl * P:(gl + 1) * P],
                            xr2[:, gi, t0:t0 + tsz],
                            start=(gi == 0), stop=(gi == CG - 1))
                    sgt = spG.tile([P, TS], BF16, tag="sgt", name="sgt")
                    nc.scalar.activation(sgt[:, :tsz], ps[:, :tsz],
                                         ACT.Sigmoid)
                    nc.sync.dma_start(out=sgdv[:, g_c, t0:t0 + tsz],
                                      in_=sgt[:, :tsz])
        psG.release()
        spG.release()
        wpG.release()
        pXr2.release()

        # ============ Phase H: FFN ============
        FBLK = min(512, Ff)
        FQ = 16 if FG >= 16 else FG
        for t0, tsz in _splits(RO, TS):
            pH = tc.alloc_tile_pool(name=f"pH{t0}", bufs=1)
            wpH = tc.alloc_tile_pool(name=f"wpH{t0}", bufs=2)
            psH = tc.alloc_tile_pool(name=f"psH{t0}", bufs=3, space="PSUM")
            psKV = tc.alloc_tile_pool(name=f"psKV{t0}", bufs=1, space="PSUM")
            kfsq = pH.tile([P, FG, TS], BF16, tag="kfsq", name="kfsq")
            # FFN1: kf = relu(xk2 @ WckT)^2
            for f0, fsz in _splits(Ff, FBLK):
                wbuf = wpH.tile([P, CG, FBLK], mm_dt, tag="wf", name="wf")
                nc.sync.dma_start(out=wbuf[:, :, :fsz],
                                  in_=wck[:, :, f0:f0 + fsz])
                for fl in range(fsz // P):
                    g_f = (f0 + fl * P) // P
                    ps = psH.tile([P, TS], F32, tag="ffn1_ps", name="ffn1_ps")
                    for gi in range(CG):
                        nc.tensor.matmul(
                            ps[:, :tsz], wbuf[:, gi, fl * P:(fl + 1) * P],
                            xk2[:, gi, t0:t0 + tsz],
                            start=(gi == 0), stop=(gi == CG - 1))
                    nc.scalar.activation(kfsq[:, g_f, :tsz], ps[:, :tsz],
                                         ACT.Relu)
                    nc.vector.tensor_tensor(kfsq[:, g_f, :tsz],
                                            kfsq[:, g_f, :tsz],
                                            kfsq[:, g_f, :tsz], ALU.mult)
            # FFN2 + final: out = x2 + sg * (kfsq @ WcvT)
            for c0, csz in _splits(Cc, CBLK):
                kvps = [psKV.tile([P, TS], F32, tag=f"kv_ps{i}",
                                  name=f"kv_ps{i}")
                        for i in range(csz // P)]
                nq = (FG + FQ - 1) // FQ
                for q in range(nq):
                    f_lo = q * FQ
                    f_n = min(FQ, FG - f_lo)
                    wbuf = wpH.tile([P, FQ, CBLK], mm_dt, tag="wf2",
                                    name="wf2")
                    nc.sync.dma_start(
                        out=wbuf[:, :f_n, :csz],
                        in_=wcv[:, f_lo:f_lo + f_n, c0:c0 + csz])
                    for gl in range(csz // P):
                        for fi in range(f_n):
                            nc.tensor.matmul(
                                kvps[gl][:, :tsz],
                                wbuf[:, fi, gl * P:(gl + 1) * P],
                                kfsq[:, f_lo + fi, :tsz],
                                start=(q == 0 and fi == 0),
                                stop=(q == nq - 1 and fi == f_n - 1))
                for gl in range(csz // P):
                    g_c = (c0 + gl * P) // P
                    x2s = wpH.tile([P, TS], F32, tag="x2s", name="x2s")
                    nc.sync.dma_start(
                        out=x2s[:, :tsz],
                        in_=x2dv[:, g_c, 1 + t0:1 + t0 + tsz])
                    sgs = wpH.tile([P, TS], BF16, tag="sgs", name="sgs")
                    nc.sync.dma_start(out=sgs[:, :tsz],
                                      in_=sgdv[:, g_c, t0:t0 + tsz])
                    ot = wpH.tile([P, TS], F32, tag="ot", name="ot")
                    nc.vector.tensor_tensor(ot[:, :tsz], sgs[:, :tsz],
                                            kvps[gl][:, :tsz], ALU.mult)
                    nc.vector.tensor_tensor(ot[:, :tsz], ot[:, :tsz],
                                            x2s[:, :tsz], ALU.add)
                    nc.sync.dma_start(out=outTv[:, g_c, t0:t0 + tsz],
                                      in_=ot[:, :tsz])
            for p in (psKV, psH, wpH, pH):
                p.release()
        pMx2.release()
        dram.release()
        const.release()

    nc.compile()
    return nc


_PROGRAM_CACHE = {}


def _get_program(key, **kw):
    if key not in _PROGRAM_CACHE:
        _PROGRAM_CACHE[key] = build_program(**kw)
    return _PROGRAM_CACHE[key]


def _host_prep(inputs, Cc=C, Dd=D_ATT, Ff=D_FFN, Bb=B, Tt=T, n_cores=N_CORES):
    """Build per-core input maps (numpy only)."""
    P = 128
    CG, DG, FG = Cc // P, Dd // P, Ff // P
    half = Tt // 2
    RO, RS, R = half, half + 1, half + 2
    bf = ml_dtypes.bfloat16

    f = {k: np.asarray(v, np.float32) for k, v in inputs.items()}
    x = f["x"]

    def swz(wT, kg):  # [K, N] -> [128, kg, N] with [p, gi, n] = wT[gi*128+p, n]
        Kdim, Ndim = wT.shape
        return np.ascontiguousarray(
            wT.reshape(kg, P, Ndim).transpose(1, 0, 2)).astype(bf)

    wk_h = swz(f["Wk"].T, CG)
    wv_h = swz(f["Wv"].T, CG)
    wr_h = swz(f["Wr"].T, CG)
    wo_h = swz(f["Wo"].T, DG)
    wck_h = swz(f["Wck"].T, CG)
    wcv_h = swz(f["Wcv"].T, FG)
    wcr_h = swz(f["Wcr"].T, CG)

    def col(v):  # [C] -> [128, CG]
        return np.ascontiguousarray(
            np.asarray(v, np.float32).reshape(-1).reshape(CG, P).T)

    ew = np.exp(-np.exp(f["time_decay"].astype(np.float64)))
    cvec_h = np.stack([
        col(f["ln1_w"]), col(f["ln1_b"]),
        col(f["tm_k"]), col(f["tm_v"]), col(f["tm_r"]),
        col(ew.astype(np.float32)), col(np.exp(f["time_first"])),
        col(f["ln2_w"]), col(f["ln2_b"]),
        col(f["cm_k"]), col(f["cm_r"]),
    ], axis=-1).astype(np.float32)  # [128, CG, 11]

    in_maps = []
    for core in range(n_cores):
        b, hh = core // 2, core % 2
        t0 = hh * half
        xr = np.zeros((R, Cc), np.float32)
        lo = t0 - 2
        src_lo = max(lo, 0)
        xr[src_lo - lo:, :] = x[b, src_lo:t0 + RO, :]
        m0 = np.full((P, 1), float(hh), np.float32)
        sel = np.zeros((P, n_cores), np.float32)
        if hh == 1:
            sel[:, core - 1] = 1.0
        in_maps.append({
            "xT": np.ascontiguousarray(xr.T),
            "wk": wk_h, "wv": wv_h, "wr": wr_h, "wo": wo_h,
            "wck": wck_h, "wcv": wcv_h, "wcr": wcr_h,
            "cvec": cvec_h, "m0": m0, "sel": sel,
        })
    return in_maps


def kernel(**inputs):
    in_maps = _host_prep(inputs)
    nc = _get_program("full")
    res = run_bass_kernel_spmd(nc, in_maps, core_ids=list(range(N_CORES)))
    half = T // 2
    out = np.empty((B, T, C), np.float32)
    for core in range(N_CORES):
        b, hh = core // 2, core % 2
        out[b, hh * half:(hh + 1) * half, :] = res.results[core]["outT"].T
    return out
